# revision 1
# baseline (speedup 1.0000x reference)
"""Multi-head self-attention (B=4, S=2048, E=1024, H=16) on 8 Trainium2 cores.

Sharding: core c -> batch c//2, head-group c%2 (8 heads each).
Each core computes a partial output for its batch (its 8 heads' contribution
through the output projection); the host sums the two partials per batch.

Per-core dataflow (everything "transposed" so softmax feeds P@V directly):
  xT(bf16) --PE--> qT,kT (d on partitions, bias added)  and  v (natural, with
  a ones column per head) --PE row-packed pairs--> scoresT (k on partitions,
  q free) in 2-bank PSUM groups --ACT exp(x/8)--> PT(bf16)
  --PE [v|ones] M=65--> unnormalized AO.T + row sums --DVE recip + GPSIMD
  partition-broadcast + DVE mul--> normalized AO.T (bf16)
  --PE--> y partial (q on partitions) --DVE +bias--> DRAM.
"""

import os
import sys

for _p in ("/opt/trn_rl_repo", "/root/.axon_site/_ro/trn_rl_repo"):
    if os.path.isdir(_p) and _p not in sys.path:
        sys.path.insert(0, _p)

import numpy as np
import ml_dtypes

import concourse.bass as bass
import concourse.mybir as mybir
import concourse.tile as tile
from concourse import bacc
from concourse.bass_utils import run_bass_kernel_spmd

BF16 = mybir.dt.bfloat16
F32 = mybir.dt.float32
EXP = mybir.ActivationFunctionType.Exp

# Full-problem constants
EMBED = 1024
NUM_HEADS = 16
HD = 64
B, S = 4, 2048
N_CORES = 8


class Dims:
    def __init__(self, s=S, e=EMBED, nh_loc=NUM_HEADS // 2):
        self.S = s                    # sequence length
        self.E = e                    # embed dim (contraction for projections)
        self.NH = nh_loc              # heads on this core
        self.EL = nh_loc * HD         # local projection output dim
        self.PAIRS = nh_loc // 2      # head pairs (row/col packing unit)
        self.NE = e // 128            # E chunks of 128
        self.NO = self.EL // 128      # output-dim chunks of 128 (== PAIRS)
        self.NQ = s // 512            # q chunks of 512
        self.NK = s // 128            # k chunks of 128
        self.NQI = 512 // 128         # q sub-chunks of 128 within a q chunk
        self.NEO = e // 512           # out-embed chunks of 512
        self.SCALE = 1.0 / np.sqrt(HD)


def _emit(nc, tc, d, t):
    """Emit the whole per-core program under TileContext tc.

    t: dict of DRAM APs (xt, wqt, wkt, wvt, wot, bq, bk, beff, y).
    """
    ctx_pools = []

    def pool(**kw):
        p = tc.tile_pool(**kw)
        pp = p.__enter__()
        ctx_pools.append(p)
        return pp

    const = pool(name="const", bufs=1)
    psA = pool(name="psA", bufs=2, space="PSUM")      # projections + out-proj
    psScore = pool(name="psScore", bufs=2, space="PSUM")
    psPV = pool(name="psPV", bufs=2, space="PSUM")
    ptP = pool(name="ptP", bufs=4)
    aotP = pool(name="aotP", bufs=8)
    recP = pool(name="recP", bufs=4)
    bcP = pool(name="bcP", bufs=4)
    yP = pool(name="yP", bufs=4)

    # ---- constant loads -------------------------------------------------
    xt_sb = []
    for e in range(d.NE):
        xx = const.tile([128, d.S], BF16, name=f"xt{e}")
        nc.sync.dma_start(xx[:], t["xt"][e * 128:(e + 1) * 128, :])
        xt_sb.append(xx)
    w_sb = {}
    for wname in ("wqt", "wkt", "wvt"):
        lst = []
        for e in range(d.NE):
            ww = const.tile([128, d.EL], BF16, name=f"{wname}{e}")
            nc.sync.dma_start(ww[:], t[wname][e * 128:(e + 1) * 128, :])
            lst.append(ww)
        w_sb[wname] = lst
    wot_sb = []
    for p in range(d.NO):
        ww = const.tile([128, d.E], BF16, name=f"wot{p}")
        nc.sync.dma_start(ww[:], t["wot"][p * 128:(p + 1) * 128, :])
        wot_sb.append(ww)
    bq_sb = const.tile([128, d.NO], F32, name="bq")
    nc.sync.dma_start(bq_sb[:], t["bq"][:])
    bk_sb = const.tile([128, d.NO], F32, name="bk")
    nc.sync.dma_start(bk_sb[:], t["bk"][:])
    beff_sb = const.tile([128, d.E], F32, name="beff")
    nc.sync.dma_start(beff_sb[:], t["beff"][:].broadcast_to([128, d.E]))

    # ---- persistent intermediate tiles ---------------------------------
    qt_sb = [const.tile([128, d.S], BF16, name=f"qt{o}") for o in range(d.NO)]
    kt_sb = [const.tile([128, d.S], BF16, name=f"kt{o}") for o in range(d.NO)]
    v_sb = [const.tile([128, d.NH * 65], BF16, name=f"v{k}") for k in range(d.NK)]

    # ---- stage A emitters ----------------------------------------------
    def emit_qk_chain(o, j, which):
        """One projection chain: 8 matmuls into a psum bank + bias-copy."""
        w = w_sb["wqt" if which == "q" else "wkt"]
        bias = bq_sb if which == "q" else bk_sb
        dst = qt_sb[o] if which == "q" else kt_sb[o]
        ps = psA.tile([128, 512], F32, name="pa", tag="pa")
        for e in range(d.NE):
            nc.tensor.matmul(
                ps[:],
                w[e][:, o * 128:(o + 1) * 128],
                xt_sb[e][:, j * 512:(j + 1) * 512],
                start=(e == 0), stop=(e == d.NE - 1),
            )
        nc.vector.tensor_scalar_add(
            dst[:, j * 512:(j + 1) * 512], ps[:], bias[:, o:o + 1]
        )

    def emit_v_sweep(sc):
        """V projection for s-chunk sc: natural layout + ones columns."""
        ps = psA.tile([128, min(512, d.EL)], F32, name="pa", tag="pa")
        for e in range(d.NE):
            nc.tensor.matmul(
                ps[:],
                xt_sb[e][:, sc * 128:(sc + 1) * 128],
                w_sb["wvt"][e][:],
                start=(e == 0), stop=(e == d.NE - 1),
            )
        vdst = v_sb[sc].rearrange("p (h m) -> p h m", h=d.NH, m=65)
        nc.vector.tensor_copy(
            vdst[:, :, 0:64],
            ps.rearrange("p (h m) -> p h m", h=d.NH, m=64),
        )
        nc.vector.memset(vdst[:, :, 64:65], 1.0)

    # fill generators: units of extra PE work to interleave into attention
    def gen_v(start_sweep):
        for sc in range(start_sweep, d.NK):
            yield lambda sc=sc: emit_v_sweep(sc)

    def gen_qk(o):
        for j in range(d.NQ):
            yield lambda j=j: emit_qk_chain(o, j, "q")
            yield lambda j=j: emit_qk_chain(o, j, "k")

    # ---- attention for one (pair, q-chunk) ------------------------------
    def attention(p, j, fill):
        hA, hB = 2 * p, 2 * p + 1
        pvA = psPV.tile([65, 512], F32, name="pv", tag="pv")
        pvB = psPV.tile([65, 512], F32, name="pv", tag="pv")
        pts = {}

        def emit_pv(g):
            nc.tensor.matmul(
                pvA[:], v_sb[g][:, hA * 65:(hA + 1) * 65], pts[g][:, 0:512],
                start=(g == 0), stop=(g == d.NK - 1),
            )
            nc.tensor.matmul(
                pvB[:], v_sb[g][:, hB * 65:(hB + 1) * 65], pts[g][:, 512:1024],
                start=(g == 0), stop=(g == d.NK - 1),
            )
            pts.pop(g)

        for g in range(d.NK):
            sc = psScore.tile([128, 1024], F32, name="score", tag="score")
            nc.tensor.matmul(
                sc[:, 0:512],
                kt_sb[p][0:64, g * 128:(g + 1) * 128],
                qt_sb[p][0:64, j * 512:(j + 1) * 512],
                start=True, stop=True, tile_position=(0, 0),
            )
            nc.tensor.matmul(
                sc[:, 512:1024],
                kt_sb[p][64:128, g * 128:(g + 1) * 128],
                qt_sb[p][64:128, j * 512:(j + 1) * 512],
                start=True, stop=True, tile_position=(64, 0),
            )
            pt = ptP.tile([128, 1024], BF16, name="pt", tag="pt")
            nc.scalar.activation(pt[:], sc[:], EXP, scale=float(d.SCALE))
            pts[g] = pt
            if g >= 1:
                emit_pv(g - 1)
            fill(g)
        emit_pv(d.NK - 1)

        aot = aotP.tile([128, 512], BF16, name="aot", tag="aot")
        for head_idx, pv in ((0, pvA), (1, pvB)):
            rec = recP.tile([1, 512], F32, name="rec", tag="rec")
            nc.vector.reciprocal(rec[:], pv[64:65, :])
            bc = bcP.tile([64, 512], F32, name="bc", tag="bc")
            nc.gpsimd.partition_broadcast(bc[:], rec[:])
            nc.vector.tensor_mul(
                aot[head_idx * 64:(head_idx + 1) * 64, :], pv[0:64, :], bc[:]
            )
        return aot

    # ---- out-projection for one q-chunk --------------------------------
    def out_proj(j, aots):
        for qi in range(d.NQI):
            for eo in range(d.NEO):
                ps = psA.tile([128, 512], F32, name="pa", tag="pa")
                for p in range(d.PAIRS):
                    nc.tensor.matmul(
                        ps[:],
                        aots[p][:, qi * 128:(qi + 1) * 128],
                        wot_sb[p][:, eo * 512:(eo + 1) * 512],
                        start=(p == 0), stop=(p == d.PAIRS - 1),
                    )
                ysb = yP.tile([128, 512], F32, name="ysb", tag="ysb")
                nc.vector.tensor_add(
                    ysb[:], ps[:], beff_sb[:, eo * 512:(eo + 1) * 512]
                )
                rows = j * 512 + qi * 128
                nc.sync.dma_start(
                    t["y"][rows:rows + 128, eo * 512:(eo + 1) * 512], ysb[:]
                )

    # ---- main schedule --------------------------------------------------
    # Pre-emit: a few V sweeps + projections for pairs 0 and 1.
    n_pre_v = min(4, d.NK)
    for sc in range(n_pre_v):
        emit_v_sweep(sc)
    for o in range(min(2, d.PAIRS)):
        for step in gen_qk(o):
            step()

    # Fill work interleaved into attention groups, keyed by (j, p).
    fills = {}
    if d.NK > n_pre_v:
        fills[(0, 0)] = gen_v(n_pre_v)
    for o in range(2, d.PAIRS):
        fills[(0, o - 1)] = gen_qk(o)

    def make_fill(j, p):
        gen = fills.get((j, p))
        if gen is None:
            return lambda g: None

        def fill(g):
            # spread the generator roughly evenly over this pair's groups
            try:
                next(gen)()
            except StopIteration:
                pass
        return fill

    for j in range(d.NQ):
        aots = []
        for p in range(d.PAIRS):
            aots.append(attention(p, j, make_fill(j, p)))
        out_proj(j, aots)

    for p in reversed(ctx_pools):
        p.__exit__(None, None, None)


def build(d=None):
    """Build + compile the Bass program. Returns (nc, names)."""
    d = d or Dims()
    nc = bacc.Bacc(
        "TRN2", target_bir_lowering=False, debug=False, num_devices=N_CORES
    )
    t = {
        "xt": nc.dram_tensor("xt", [d.E, d.S], BF16, kind="ExternalInput").ap(),
        "wqt": nc.dram_tensor("wqt", [d.E, d.EL], BF16, kind="ExternalInput").ap(),
        "wkt": nc.dram_tensor("wkt", [d.E, d.EL], BF16, kind="ExternalInput").ap(),
        "wvt": nc.dram_tensor("wvt", [d.E, d.EL], BF16, kind="ExternalInput").ap(),
        "wot": nc.dram_tensor("wot", [d.EL, d.E], BF16, kind="ExternalInput").ap(),
        "bq": nc.dram_tensor("bq", [128, d.NO], F32, kind="ExternalInput").ap(),
        "bk": nc.dram_tensor("bk", [128, d.NO], F32, kind="ExternalInput").ap(),
        "beff": nc.dram_tensor("beff", [1, d.E], F32, kind="ExternalInput").ap(),
        "y": nc.dram_tensor("y", [d.S, d.E], F32, kind="ExternalOutput").ap(),
    }
    with tile.TileContext(nc) as tc:
        _emit(nc, tc, d, t)
    nc.compile()
    return nc


def make_in_maps(x, w_q, b_q, w_k, b_k, w_v, b_v, w_out, b_out, d=None):
    """Per-core input dicts for the full problem."""
    d = d or Dims()
    bf = ml_dtypes.bfloat16
    in_maps = []
    xt_cache = {}
    beff_full = (b_out + w_out @ b_v).astype(np.float32)
    for c in range(N_CORES):
        b = c // 2
        hg = c % 2
        rows = slice(hg * d.EL, (hg + 1) * d.EL)
        if b not in xt_cache:
            xt_cache[b] = np.ascontiguousarray(x[b].T).astype(bf)
        in_maps.append({
            "xt": xt_cache[b],
            "wqt": np.ascontiguousarray(w_q[rows].T).astype(bf),
            "wkt": np.ascontiguousarray(w_k[rows].T).astype(bf),
            "wvt": np.ascontiguousarray(w_v[rows].T).astype(bf),
            "wot": np.ascontiguousarray(w_out[:, rows].T).astype(bf),
            "bq": np.ascontiguousarray(
                b_q[rows].reshape(d.NO, 128).T).astype(np.float32),
            "bk": np.ascontiguousarray(
                b_k[rows].reshape(d.NO, 128).T).astype(np.float32),
            "beff": (beff_full if hg == 0 else
                     np.zeros_like(beff_full)).reshape(1, d.E),
        })
    return in_maps


_NC_CACHE = None


def _get_nc():
    global _NC_CACHE
    if _NC_CACHE is None:
        _NC_CACHE = build()
    return _NC_CACHE


def run(inputs, trace=False, **spmd_kwargs):
    nc = _get_nc()
    in_maps = make_in_maps(**inputs)
    res = run_bass_kernel_spmd(
        nc, in_maps, list(range(N_CORES)), trace=trace, **spmd_kwargs
    )
    y = np.empty((B, S, EMBED), dtype=np.float32)
    for b in range(B):
        y[b] = res.results[2 * b]["y"] + res.results[2 * b + 1]["y"]
    return y, res


def kernel(**inputs):
    y, _ = run(inputs)
    return y


# revision 9
# speedup vs baseline: 1.0496x; 1.0496x over previous
"""Multi-head self-attention (B=4, S=2048, E=1024, H=16) on 8 Trainium2 cores.

Sharding: core c -> batch c//2, head-group c%2 (8 heads each).
Each core computes a partial output for its batch (its 8 heads' contribution
through the output projection); the host sums the two partials per batch.

Per-core dataflow (everything "transposed" so softmax feeds P@V directly):
  xT(bf16) --PE--> qT,kT (d on partitions, bias added)  and  v (natural, with
  a ones column per head) --PE row-packed pairs--> scoresT (k on partitions,
  q free) in 2-bank PSUM groups --ACT exp(x/8)--> PT(bf16)
  --PE [v|ones] M=65--> unnormalized AO.T + row sums --DVE recip + GPSIMD
  partition-broadcast + DVE mul--> normalized AO.T (bf16)
  --PE--> y partial (q on partitions) --DVE +bias--> DRAM.
"""

import os
import sys

for _p in ("/opt/trn_rl_repo", "/root/.axon_site/_ro/trn_rl_repo"):
    if os.path.isdir(_p) and _p not in sys.path:
        sys.path.insert(0, _p)

import numpy as np
import ml_dtypes

import concourse.bass as bass
import concourse.mybir as mybir
import concourse.tile as tile
from concourse import bacc
from concourse.bass_utils import run_bass_kernel_spmd

BF16 = mybir.dt.bfloat16
F32 = mybir.dt.float32
EXP = mybir.ActivationFunctionType.Exp

# Full-problem constants
EMBED = 1024
NUM_HEADS = 16
HD = 64
B, S = 4, 2048
N_CORES = 8


class Dims:
    def __init__(self, s=S, e=EMBED, nh_loc=NUM_HEADS // 2):
        self.S = s                    # sequence length
        self.E = e                    # embed dim (contraction for projections)
        self.NH = nh_loc              # heads on this core
        self.EL = nh_loc * HD         # local projection output dim
        self.PAIRS = nh_loc // 2      # head pairs (row/col packing unit)
        self.NE = e // 128            # E chunks of 128
        self.NO = self.EL // 128      # output-dim chunks of 128 (== PAIRS)
        self.NQ = s // 512            # q chunks of 512
        self.NK = s // 128            # k chunks of 128
        self.NQI = 512 // 128         # q sub-chunks of 128 within a q chunk
        self.NEO = e // 512           # out-embed chunks of 512
        self.SCALE = 1.0 / np.sqrt(HD)


def _emit(nc, tc, d, t):
    """Emit the whole per-core program under TileContext tc.

    t: dict of DRAM APs (xt, wqt, wkt, wvt, wot, bq, bk, beff, y).
    """
    ctx_pools = []

    def pool(**kw):
        p = tc.tile_pool(**kw)
        pp = p.__enter__()
        ctx_pools.append(p)
        return pp

    const = pool(name="const", bufs=1)
    psA = pool(name="psA", bufs=2, space="PSUM")      # projections + out-proj
    psScore = pool(name="psScore", bufs=2, space="PSUM")
    psPV = pool(name="psPV", bufs=2, space="PSUM")
    ptP = pool(name="ptP", bufs=4)
    aotP = pool(name="aotP", bufs=8)
    recP = pool(name="recP", bufs=4)
    bcP = pool(name="bcP", bufs=4)
    yP = pool(name="yP", bufs=4)

    # ---- constant loads -------------------------------------------------
    # Load order tracks first use: the V sweeps and the first q/k chains
    # consume (wvt[e], xt[e], wqt[e]) chunk by chunk starting ~1us in, so
    # interleave those; everything else follows.
    w_sb = {"wvt": [], "wqt": [], "wkt": []}
    xt_sb = []
    bq_sb = const.tile([128, d.NO], F32, name="bq")
    bk_sb = const.tile([128, d.NO], F32, name="bk")
    for e in range(d.NE):
        ww = const.tile([128, d.EL], BF16, name=f"wvt{e}")
        nc.sync.dma_start(ww[:], t["wvt"][e * 128:(e + 1) * 128, :])
        w_sb["wvt"].append(ww)
        xx = const.tile([128, d.S], BF16, name=f"xt{e}")
        nc.sync.dma_start(xx[:], t["xt"][e * 128:(e + 1) * 128, :])
        xt_sb.append(xx)
        ww = const.tile([128, d.EL], BF16, name=f"wqt{e}")
        nc.sync.dma_start(ww[:], t["wqt"][e * 128:(e + 1) * 128, :])
        w_sb["wqt"].append(ww)
        if e == 0:
            nc.sync.dma_start(bq_sb[:], t["bq"][:])
            nc.sync.dma_start(bk_sb[:], t["bk"][:])
    for e in range(d.NE):
        ww = const.tile([128, d.EL], BF16, name=f"wkt{e}")
        nc.sync.dma_start(ww[:], t["wkt"][e * 128:(e + 1) * 128, :])
        w_sb["wkt"].append(ww)
    wot_sb = []
    for p in range(d.NO):
        ww = const.tile([128, d.E], BF16, name=f"wot{p}")
        nc.sync.dma_start(ww[:], t["wot"][p * 128:(p + 1) * 128, :])
        wot_sb.append(ww)
    beff_sb = const.tile([128, d.E], F32, name="beff")
    nc.sync.dma_start(beff_sb[:], t["beff"][:].broadcast_to([128, d.E]))

    # ---- persistent intermediate tiles ---------------------------------
    qt_sb = [const.tile([128, d.S], BF16, name=f"qt{o}") for o in range(d.NO)]
    kt_sb = [const.tile([128, d.S], BF16, name=f"kt{o}") for o in range(d.NO)]
    v_sb = [const.tile([128, d.NH * 65], BF16, name=f"v{k}") for k in range(d.NK)]

    # ---- stage A emitters ----------------------------------------------
    def emit_qk_chain(o, j, which):
        """One projection chain: 8 matmuls into a psum bank + bias-copy."""
        w = w_sb["wqt" if which == "q" else "wkt"]
        bias = bq_sb if which == "q" else bk_sb
        dst = qt_sb[o] if which == "q" else kt_sb[o]
        ps = psA.tile([128, 512], F32, name="pa", tag="pa")
        for e in range(d.NE):
            nc.tensor.matmul(
                ps[:],
                w[e][:, o * 128:(o + 1) * 128],
                xt_sb[e][:, j * 512:(j + 1) * 512],
                start=(e == 0), stop=(e == d.NE - 1),
            )
        nc.vector.tensor_scalar_add(
            dst[:, j * 512:(j + 1) * 512], ps[:], bias[:, o:o + 1]
        )

    def emit_v_sweep(sc):
        """V projection for s-chunk sc: natural layout + ones columns."""
        ps = psA.tile([128, min(512, d.EL)], F32, name="pa", tag="pa")
        for e in range(d.NE):
            nc.tensor.matmul(
                ps[:],
                xt_sb[e][:, sc * 128:(sc + 1) * 128],
                w_sb["wvt"][e][:],
                start=(e == 0), stop=(e == d.NE - 1),
            )
        vdst = v_sb[sc].rearrange("p (h m) -> p h m", h=d.NH, m=65)
        nc.vector.tensor_copy(
            vdst[:, :, 0:64],
            ps.rearrange("p (h m) -> p h m", h=d.NH, m=64),
        )
        nc.vector.memset(vdst[:, :, 64:65], 1.0)

    # fill generators: units of extra PE work to interleave into attention
    def gen_v(start_sweep):
        for sc in range(start_sweep, d.NK):
            yield lambda sc=sc: emit_v_sweep(sc)

    def gen_qk(o):
        for j in range(d.NQ):
            yield lambda j=j: emit_qk_chain(o, j, "q")
            yield lambda j=j: emit_qk_chain(o, j, "k")

    # ---- attention for one (pair, q-chunk) ------------------------------
    def attention(p, j, fill):
        hA, hB = 2 * p, 2 * p + 1
        pvA = psPV.tile([65, 512], F32, name="pv", tag="pv")
        pvB = psPV.tile([65, 512], F32, name="pv", tag="pv")
        pts = {}

        def emit_pv(g):
            nc.tensor.matmul(
                pvA[:], v_sb[g][:, hA * 65:(hA + 1) * 65], pts[g][:, 0:512],
                start=(g == 0), stop=(g == d.NK - 1),
            )
            nc.tensor.matmul(
                pvB[:], v_sb[g][:, hB * 65:(hB + 1) * 65], pts[g][:, 512:1024],
                start=(g == 0), stop=(g == d.NK - 1),
            )
            pts.pop(g)

        for g in range(d.NK):
            sc = psScore.tile([128, 1024], F32, name="score", tag="score")
            nc.tensor.matmul(
                sc[:, 0:512],
                kt_sb[p][0:64, g * 128:(g + 1) * 128],
                qt_sb[p][0:64, j * 512:(j + 1) * 512],
                start=True, stop=True, tile_position=(0, 0),
            )
            nc.tensor.matmul(
                sc[:, 512:1024],
                kt_sb[p][64:128, g * 128:(g + 1) * 128],
                qt_sb[p][64:128, j * 512:(j + 1) * 512],
                start=True, stop=True, tile_position=(64, 0),
            )
            pt = ptP.tile([128, 1024], BF16, name="pt", tag="pt")
            nc.scalar.activation(pt[:], sc[:], EXP, scale=float(d.SCALE))
            pts[g] = pt
            if g >= 1:
                emit_pv(g - 1)
            fill(g)
        emit_pv(d.NK - 1)

        # Copy PSUM out on DVE right away so the PV banks free for the next
        # pair; the recip/broadcast/normalize chain then runs from SBUF.
        aot = aotP.tile([128, 512], BF16, name="aot", tag="aot")
        for head_idx, pv in ((0, pvA), (1, pvB)):
            aou = recP.tile([65, 512], F32, name="aou", tag="aou")
            nc.vector.tensor_copy(aou[:], pv[:])
            rec = recP.tile([1, 512], F32, name="rec", tag="rec")
            nc.vector.reciprocal(rec[:], aou[64:65, :])
            bc = bcP.tile([64, 512], F32, name="bc", tag="bc")
            nc.gpsimd.partition_broadcast(bc[:], rec[:])
            nc.vector.tensor_mul(
                aot[head_idx * 64:(head_idx + 1) * 64, :], aou[0:64, :], bc[:]
            )
        return aot

    # ---- out-projection for one q-chunk --------------------------------
    def out_proj_sweep(j, aots, qi, eo):
        ps = psA.tile([128, 512], F32, name="pa", tag="pa")
        for p in range(d.PAIRS):
            nc.tensor.matmul(
                ps[:],
                aots[p][:, qi * 128:(qi + 1) * 128],
                wot_sb[p][:, eo * 512:(eo + 1) * 512],
                start=(p == 0), stop=(p == d.PAIRS - 1),
            )
        ysb = yP.tile([128, 512], F32, name="ysb", tag="ysb")
        nc.vector.tensor_add(
            ysb[:], ps[:], beff_sb[:, eo * 512:(eo + 1) * 512]
        )
        rows = j * 512 + qi * 128
        nc.sync.dma_start(
            t["y"][rows:rows + 128, eo * 512:(eo + 1) * 512], ysb[:]
        )

    def gen_out_proj(j, aots):
        for qi in range(d.NQI):
            for eo in range(d.NEO):
                yield lambda qi=qi, eo=eo: out_proj_sweep(j, aots, qi, eo)

    # ---- main schedule --------------------------------------------------
    # ACT (exp) is the steady-state bottleneck, so attention groups start as
    # early as possible and all other PE work (V sweeps, remaining q/k
    # projection chains, out-projections) is drip-fed between groups from a
    # single fill queue, ordered by first use.  Tile dependencies keep this
    # correct regardless of pacing; the ordering only shapes the overlap.
    import heapq
    from collections import deque

    # Pre-emit just what attention(p0, j0) needs up front.
    n_pre_v = min(2, d.NK)
    for sc in range(n_pre_v):
        emit_v_sweep(sc)
    emit_qk_chain(0, 0, "q")
    emit_qk_chain(0, 0, "k")

    v_queue = deque(range(n_pre_v, d.NK))   # V sweeps, JIT inside span 0
    work_q = []                              # heap of (deadline_span, seq, fn)
    seq_ctr = [0]

    def push(deadline, fn):
        heapq.heappush(work_q, (deadline, seq_ctr[0], fn))
        seq_ctr[0] += 1

    for p in range(d.PAIRS):
        for j in range(d.NQ):
            if p == 0 and j == 0:
                continue
            # Emission order defines Tile deps, so a chain must be emitted
            # before the first attention span that reads it.  Q chain (p, j)
            # is read only by span (j, p); the K chain (p, j) is read by
            # EVERY span of pair p (attention scans the full key sequence),
            # so it is due before pair p's first span.
            push(j * d.PAIRS + p, lambda p=p, j=j: emit_qk_chain(p, j, "q"))
            push(p, lambda p=p, j=j: emit_qk_chain(p, j, "k"))

    def make_fill(span):
        def fill(g):
            if v_queue:
                emit_v_sweep(v_queue.popleft())
            elif work_q and work_q[0][0] <= span + 2:
                heapq.heappop(work_q)[2]()
        return fill

    for j in range(d.NQ):
        aots = []
        for p in range(d.PAIRS):
            span = j * d.PAIRS + p
            # correctness: everything due by this span must be emitted now
            while work_q and work_q[0][0] <= span:
                heapq.heappop(work_q)[2]()
            aots.append(attention(p, j, make_fill(span)))
        for step in gen_out_proj(j, aots):
            push(j * d.PAIRS + d.PAIRS, step)
    while v_queue:
        emit_v_sweep(v_queue.popleft())
    while work_q:
        heapq.heappop(work_q)[2]()

    for p in reversed(ctx_pools):
        p.__exit__(None, None, None)


def build(d=None):
    """Build + compile the Bass program. Returns (nc, names)."""
    d = d or Dims()
    nc = bacc.Bacc(
        "TRN2", target_bir_lowering=False, debug=False, num_devices=N_CORES
    )
    t = {
        "xt": nc.dram_tensor("xt", [d.E, d.S], BF16, kind="ExternalInput").ap(),
        "wqt": nc.dram_tensor("wqt", [d.E, d.EL], BF16, kind="ExternalInput").ap(),
        "wkt": nc.dram_tensor("wkt", [d.E, d.EL], BF16, kind="ExternalInput").ap(),
        "wvt": nc.dram_tensor("wvt", [d.E, d.EL], BF16, kind="ExternalInput").ap(),
        "wot": nc.dram_tensor("wot", [d.EL, d.E], BF16, kind="ExternalInput").ap(),
        "bq": nc.dram_tensor("bq", [128, d.NO], F32, kind="ExternalInput").ap(),
        "bk": nc.dram_tensor("bk", [128, d.NO], F32, kind="ExternalInput").ap(),
        "beff": nc.dram_tensor("beff", [1, d.E], F32, kind="ExternalInput").ap(),
        "y": nc.dram_tensor("y", [d.S, d.E], F32, kind="ExternalOutput").ap(),
    }
    with tile.TileContext(nc) as tc:
        _emit(nc, tc, d, t)
    nc.compile()
    return nc


def make_in_maps(x, w_q, b_q, w_k, b_k, w_v, b_v, w_out, b_out, d=None):
    """Per-core input dicts for the full problem."""
    d = d or Dims()
    bf = ml_dtypes.bfloat16
    in_maps = []
    xt_cache = {}
    beff_full = (b_out + w_out @ b_v).astype(np.float32)
    for c in range(N_CORES):
        b = c // 2
        hg = c % 2
        rows = slice(hg * d.EL, (hg + 1) * d.EL)
        if b not in xt_cache:
            xt_cache[b] = np.ascontiguousarray(x[b].T).astype(bf)
        in_maps.append({
            "xt": xt_cache[b],
            "wqt": np.ascontiguousarray(w_q[rows].T).astype(bf),
            "wkt": np.ascontiguousarray(w_k[rows].T).astype(bf),
            "wvt": np.ascontiguousarray(w_v[rows].T).astype(bf),
            "wot": np.ascontiguousarray(w_out[:, rows].T).astype(bf),
            "bq": np.ascontiguousarray(
                b_q[rows].reshape(d.NO, 128).T).astype(np.float32),
            "bk": np.ascontiguousarray(
                b_k[rows].reshape(d.NO, 128).T).astype(np.float32),
            "beff": (beff_full if hg == 0 else
                     np.zeros_like(beff_full)).reshape(1, d.E),
        })
    return in_maps


_NC_CACHE = None


def _get_nc():
    global _NC_CACHE
    if _NC_CACHE is None:
        _NC_CACHE = build()
    return _NC_CACHE


def run(inputs, trace=False, **spmd_kwargs):
    nc = _get_nc()
    in_maps = make_in_maps(**inputs)
    res = run_bass_kernel_spmd(
        nc, in_maps, list(range(N_CORES)), trace=trace, **spmd_kwargs
    )
    y = np.empty((B, S, EMBED), dtype=np.float32)
    for b in range(B):
        y[b] = res.results[2 * b]["y"] + res.results[2 * b + 1]["y"]
    return y, res


def kernel(**inputs):
    y, _ = run(inputs)
    return y
